# revision 65
# baseline (speedup 1.0000x reference)
"""Multi-head self-attention (B=2, L=2048, D=1024, H=16, hd=64) on 8 trn2 cores.

Sharding: core c = 4*b + g  (b = batch, g = head-group of 4 heads).
Each core computes Q/K/V projections for its 256 hidden dims (4 heads),
attention for those heads, and a partial output projection
(ctx_g @ Wo[:, g-slice].T + bo/4).  Host sums the 4 partials per batch.

Device algorithm (per core):
  - Inputs arrive pre-transposed from host: xT [1024, 2048] (d-major),
    WqT/WkT/WvT [1024, 256], WoT [256, 1024], all bf16 (halves the input
    DMA). Q^T/K^T are kept in float32r (full fp32 bits; the PE streams
    1 cycle/row vs 4 for float32) so the scores feeding exp stay sharp;
    P/V/ctx are bf16.
  - Weight DMAs are issued lazily right before their first consumer
    (consumers wait on the cumulative DMA count, so unrelated earlier
    DMAs delay them), and PE warmup matmuls run under the input-DMA
    window to hold the HAM clock gate open.
  - QT/KT = W.T-projections in [m, L] layout (m on partitions) so that
    S^T = K Q^T comes straight out of the PE per (k-tile, q-bank) with
    k on partitions and q on the free dim. Head pairs ride PE row groups
    0-63/64-127 (hh-interleaved emission for row-group concurrency).
    Q/K biases ride the DVE evacuation as tensor_scalar adds.
  - P^T = exp(S^T/8) on the scalar engine (PSUM -> bf16 SBUF), grouped 3
    k-tiles per activation op to amortize the ~350-cycle op overhead.
  - ctx^T = [V | 1].T-weighted PV matmul (bf16): the appended ones column
    makes PSUM row 64 the softmax denominator for each q.
  - Software pipelining: PV lags scores+exp by one group; the tiny tail
    group (kt 15) is scored first per (j,qb); projections and output-
    projection chunks slot between groups as PE filler while exp runs.
  - Normalization: recip(denoms) -> rank-1 matmul broadcast -> multiply
    during PSUM evacuation (DVE).
  - Output projection + bias via DVE add during evacuation (V bias too).
"""

import os
import sys

import numpy as np

for _p in ("/opt/trn_rl_repo", "/root/.axon_site/_ro/trn_rl_repo"):
    if os.path.isdir(_p) and _p not in sys.path:
        sys.path.insert(0, _p)

L = 2048
D = 1024
HD = 64
H_LOC = 4  # heads per core
M_LOC = H_LOC * HD  # 256 hidden dims per core
N_CORES = 8
KT_TILES = L // 128  # 16 k tiles
QB = L // 512  # 4 q banks
DT_TILES = D // 128  # 8 contraction tiles for projections

_PROG = None
_PROG_UNSPLIT = None
LAST_RESULTS = None  # BassKernelResults of the most recent HW run


def _build_program(split=True, reps=1):
    import concourse.bass as bass
    import concourse.mybir as mybir
    import concourse.tile as tile

    fp32 = mybir.dt.float32
    f32r = mybir.dt.float32r
    bf16 = mybir.dt.bfloat16
    Exp = mybir.ActivationFunctionType.Exp

    # Matmul dtypes: float32r (same fp32 bit layout, 1 PE cycle/row vs 4 for
    # float32 at moving free dim >= 256) for the precision-sensitive Q/K path
    # (scores feed exp); bf16 for x/weights/P/V/ctx where rounding is benign.
    # Every fp32r matmul operand must be PRODUCED as fp32r (DMA, DVE copy,
    # scalar activation all qualify) or the BIR verifier rejects the program.
    nc = bass.Bass()

    xta = nc.dram_tensor("xta", [D, L], bf16, kind="ExternalInput")
    wqa = nc.dram_tensor("wqa", [D, M_LOC], bf16, kind="ExternalInput")
    wka = nc.dram_tensor("wka", [D, M_LOC], bf16, kind="ExternalInput")
    wva = nc.dram_tensor("wva", [D, M_LOC], bf16, kind="ExternalInput")
    wqb = nc.dram_tensor("wqb", [128, 2], f32r, kind="ExternalInput")
    wkb = nc.dram_tensor("wkb", [128, 2], f32r, kind="ExternalInput")
    wvb = nc.dram_tensor("wvb", [128, M_LOC], f32r, kind="ExternalInput")
    woa = nc.dram_tensor("woa", [M_LOC, D], bf16, kind="ExternalInput")
    wob = nc.dram_tensor("wob", [128, D], f32r, kind="ExternalInput")
    outp = nc.dram_tensor("outp", [L, D], fp32, kind="ExternalOutput")

    with nc.allow_low_precision(reason="float32r is fp32-width; rounding loss is negligible"), tile.TileContext(nc) as tc:
        with (
            tc.tile_pool(name="const", bufs=1) as cpool,
            tc.tile_pool(name="pt", bufs=2) as ptpool,
            tc.tile_pool(name="ev", bufs=2) as epool,
            tc.tile_pool(name="psum", bufs=2, space="PSUM") as psum,
        ):
            # ---- persistent SBUF tiles ----
            # weights live as one [128, dt, M_LOC] tile each: one DMA per
            # matrix (DMA dispatch serializes at ~650ns/instruction)
            wq_t = cpool.tile([128, DT_TILES, M_LOC], bf16, tag="wq", name="wq")
            wk_t = cpool.tile([128, DT_TILES, M_LOC], bf16, tag="wk", name="wk")
            wv_t = cpool.tile([128, DT_TILES, M_LOC], bf16, tag="wv", name="wv")
            wq1 = cpool.tile([128, 2], f32r, tag="wqbias", name="wqbias")
            wk1 = cpool.tile([128, 2], f32r, tag="wkbias", name="wkbias")
            wv1 = cpool.tile([128, M_LOC], f32r, tag="wvbias", name="wvbias")
            wo_t = [cpool.tile([128, D], bf16, tag=f"wo{j}", name=f"wo{j}") for j in range(2)]
            wo1 = cpool.tile([128, D], f32r, tag="wobias", name="wobias")
            qt_t = {}  # (j, lb) -> Q^T [m-tile 128, 512]
            kt_t = {}
            for j in range(2):
                for lb in range(QB):
                    qt_t[(j, lb)] = cpool.tile([128, 512], f32r, tag=f"qt{j}_{lb}", name=f"qt{j}_{lb}")
                    kt_t[(j, lb)] = cpool.tile([128, 512], f32r, tag=f"kt{j}_{lb}", name=f"kt{j}_{lb}")
            # V with appended ones column: [l-part, h, 65]
            v_t = [cpool.tile([128, H_LOC, HD + 1], bf16, tag=f"v{lt}", name=f"v{lt}")
                   for lt in range(KT_TILES)]
            ctxn = {}  # (j, qb) -> normalized ctx^T [128 m, 512 q]
            for j in range(2):
                for qb in range(QB):
                    ctxn[(j, qb)] = cpool.tile([128, 512], bf16, tag=f"cn{j}_{qb}", name=f"cn{j}_{qb}")
            ones = cpool.tile([128, 512], f32r, tag="ones", name="ones")[0:1, :]
            warm = cpool.tile([128, 8], fp32, tag="warm", name="warm")[0:1, :]

            # ---- constants / warmup ----
            nc.vector.memset(ones[:].bitcast(fp32), 1.0)
            for lt in range(KT_TILES):
                nc.vector.memset(v_t[lt][:, :, HD:HD + 1], 1.0)
            # trigger the exp table load early (hides under input DMA)
            nc.scalar.activation(out=warm[:], in_=ones[0:1, 0:8].bitcast(fp32), func=Exp)
            # PE warmup under the input-DMA window: keeps the HAM activity
            # monitor busy so the real matmuls start at 2.4 GHz instead of
            # paying the cold 1.2 GHz window. Reuses the ctx PSUM tag (no
            # extra bank) and is done long before the first PV needs it.
            wps = cpool_warm = psum.tile([1, 512], fp32, tag="ctx", name="warmps")
            for _ in range(16):
                nc.tensor.matmul(
                    wps[:], ones[0:1, 0:1], ones[0:1, :], start=True, stop=True
                )

            # ---- emission helpers ----
            xt_blocks = {}

            xta_r = xta[:].rearrange("(dt p) l -> p dt l", p=128)

            def alloc_xt_block(lb):
                """One DMA brings the whole 512-wide L chunk of x^T as a
                [128, dt, 512] tile (kept resident so the j=1 projection
                pass reuses it)."""
                if lb in xt_blocks:
                    return xt_blocks[lb]
                t = ptpool.tile([128, DT_TILES, 512], bf16, tag="xt", name="xtb",
                                bufs=4)
                nc.sync.dma_start(t[:], xta_r[:, :, lb * 512:(lb + 1) * 512])
                xt_blocks[lb] = t
                return t

            # ---- weight DMAs, issued lazily: consumers wait on the
            # CUMULATIVE dma count, so a DMA emitted before an instruction
            # delays it even if unrelated. Each weight group is issued right
            # before its first consumer. ----
            wv1_t, wo1_t = wv1, wo1
            wv1, wo1 = wv1[0:1, :], wo1[0:1, :]
            _w_issued = set()

            def ensure_w(which):
                if which in _w_issued:
                    return
                _w_issued.add(which)
                if which == "k":
                    nc.sync.dma_start(wk1[:], wkb[:])
                    nc.sync.dma_start(
                        wk_t[:], wka[:].rearrange("(dt p) m -> p dt m", p=128))
                elif which == "q":
                    nc.sync.dma_start(wq1[:], wqb[:])
                    nc.sync.dma_start(
                        wq_t[:], wqa[:].rearrange("(dt p) m -> p dt m", p=128))
                elif which == "v":
                    nc.sync.dma_start(wv1_t[:], wvb[:])
                    nc.sync.dma_start(
                        wv_t[:], wva[:].rearrange("(dt p) m -> p dt m", p=128))
                elif which == "o":
                    nc.sync.dma_start(wo1_t[:], wob[:])
                    for j in range(2):
                        nc.sync.dma_start(wo_t[j][:], woa[j * 128:(j + 1) * 128, :])

            def emit_qk_group(dst, w_tiles, w1, j, xt_blk):
                """dst[m, l] = sum_d W^T[d, m] x^T[d, l] + b[m]  (one q/k bank).

                The bias (constant along l, per-partition in m) rides the DVE
                evacuation as a tensor_scalar add instead of a rank-1 matmul.
                """
                ps = psum.tile([128, 512], fp32, tag="st", name="st")
                for dt in range(DT_TILES):
                    nc.tensor.matmul(
                        ps[:],
                        (w_tiles[:, dt, j * 128:(j + 1) * 128]),
                        (xt_blk[:, dt, :]),
                        start=(dt == 0),
                        stop=(dt == DT_TILES - 1),
                    )
                nc.vector.tensor_scalar_add(
                    out=dst[:], in0=ps[:], scalar1=w1[:, j:j + 1].bitcast(fp32)
                )

            def emit_v_group(lt, xt_blk):
                """v_t[lt][l, h, d] = sum_d' x^T[d', l] Wv^T[d', (h,d)] + bv."""
                li = lt % 4
                ps = psum.tile([128, M_LOC], fp32, tag="st", name="st")
                for dt in range(DT_TILES):
                    nc.tensor.matmul(
                        ps[:],
                        (xt_blk[:, dt, li * 128:(li + 1) * 128]),
                        (wv_t[:, dt, :]),
                        start=(dt == 0),
                        stop=(dt == DT_TILES - 1),
                    )
                # bias rides the DVE evacuation (wvb rows are all identical)
                nc.vector.tensor_add(
                    out=v_t[lt][:, :, 0:HD],
                    in0=ps.rearrange("p (h d) -> p h d", d=HD),
                    in1=wv1_t[:].bitcast(fp32).rearrange("p (h d) -> p h d", d=HD),
                )

            # kt-groups per (j, qb): sizes 3,3,3,3,3,1 (st slot = 3 banks)
            GROUPS = [(0, 3), (3, 3), (6, 3), (9, 3), (12, 3), (15, 1)]

            def emit_scores(j, qb, k0, gn):
                """S^T -> exp for kt in [k0, k0+gn); returns the PV work item."""
                sts, pts = [], []
                for hh in range(2):
                    sts.append(psum.tile([128, 3, 512], fp32, tag="st", name="st"))
                # hh-interleaved emission: consecutive matmuls target disjoint
                # PE row groups (rows 0-63 vs 64-127), so on HW each can start
                # ~4ns after the previous one -> the head pair runs concurrent.
                for u in range(gn):
                    kt = k0 + u
                    for hh in range(2):
                        r0, r1 = hh * HD, (hh + 1) * HD
                        nc.tensor.matmul(
                            sts[hh][:, u, :],
                            (kt_t[(j, kt // 4)][r0:r1, (kt % 4) * 128:(kt % 4 + 1) * 128]),
                            (qt_t[(j, qb)][r0:r1, :]),
                            start=True,
                            stop=True,
                        )
                for hh in range(2):
                    # bufs=3: G5 is scored first and its pts stay live across
                    # the whole qb while the main pipeline double-buffers
                    pt = ptpool.tile([128, 3, 512], bf16, tag="pt", name="pt", bufs=6)
                    pts.append(pt)
                    if os.environ.get("KABL_NOEXP"):
                        nc.vector.tensor_copy(out=pt[:, 0:gn, :], in_=sts[hh][:, 0:gn, :])
                    else:
                        nc.scalar.activation(
                            out=pt[:, 0:gn, :], in_=sts[hh][:, 0:gn, :],
                            func=Exp, scale=0.125,
                        )
                return (pts, k0, gn)

            def emit_pv(j, ctx_ab, work):
                pts, k0, gn = work
                for u in range(gn):
                    kt = k0 + u
                    for hh in range(2):
                        nc.tensor.matmul(
                            ctx_ab[hh][:],
                            (v_t[kt][:, 2 * j + hh, :]),
                            (pts[hh][:, u, :]),
                            start=(kt == 0),
                            stop=(kt == KT_TILES - 1),
                        )

            def emit_attn_epilogue(j, qb, ctx_ab):
                for hh in range(2):
                    rec = epool.tile([1, 512], f32r, tag="rec", name="rec")
                    nc.vector.reciprocal(rec[:], ctx_ab[hh][HD:HD + 1, :])
                    rp = psum.tile([HD, 512], fp32, tag="st", name="st")
                    nc.tensor.matmul(
                        rp[:], (ones[0:1, 0:HD]), (rec[:]), start=True, stop=True
                    )
                    # NB: one DVE op must not read two PSUM operands (single
                    # PSUM port) — stage rp through SBUF.
                    rsb = epool.tile([HD, 512], fp32, tag="rsb", name="rsb")
                    nc.vector.tensor_copy(out=rsb[:], in_=rp[:])
                    nc.vector.tensor_mul(
                        out=ctxn[(j, qb)][hh * HD:(hh + 1) * HD, :],
                        in0=ctx_ab[hh][0:HD, :],
                        in1=rsb[:],
                    )

            def emit_oproj_chunk(qb, qi):
                qt = qb * 4 + qi
                ot = epool.tile([128, 2, 512], fp32, tag="ot", name="ot")
                for nb in range(2):
                    ps = psum.tile([128, 512], fp32, tag="st", name="st")
                    for j in range(2):
                        nc.tensor.matmul(
                            ps[:],
                            (ctxn[(j, qb)][:, qi * 128:(qi + 1) * 128]),
                            (wo_t[j][:, nb * 512:(nb + 1) * 512]),
                            start=(j == 0),
                            stop=(j == 1),
                        )
                    # bias (0.25*bo, rows identical) rides the DVE evacuation
                    nc.vector.tensor_add(
                        out=ot[:, nb, :], in0=ps[:],
                        in1=wo1_t[:, nb * 512:(nb + 1) * 512].bitcast(fp32),
                    )
                # one [128, 1024] store per chunk instead of two halves
                if not os.environ.get("KABL_NOOUT"):
                    nc.sync.dma_start(
                        outp[qt * 128:(qt + 1) * 128, :],
                        ot.rearrange("p a b -> p (a b)"),
                    )

            # ---- emission schedule ----
            # Software-pipelined: PV lags scores+exp by one kt-group so the PE
            # always has independent work (next scores, projections, oproj
            # chunks) in its stream while the scalar engine runs exp.
            def alloc_ctx():
                # the first PV matmul (kt==0) opens the accumulation group
                # with start=True; surplus waits are NOP-split by
                # _split_matmul_waits.
                return [psum.tile([HD + 1, 512], fp32, tag="ctx", name="ctx")
                        for _ in range(2)]

            def lb_parts(j, lb, with_v):
                """Projection work for one L-chunk as filler thunks."""
                xt_blk = alloc_xt_block(lb)
                parts = [
                    lambda: emit_qk_group(kt_t[(j, lb)], wk_t, wk1, j, xt_blk),
                    lambda: emit_qk_group(qt_t[(j, lb)], wq_t, wq1, j, xt_blk),
                ]
                if with_v:
                    for lt in range(lb * 4, lb * 4 + 4):
                        parts.append(lambda lt=lt: emit_v_group(lt, xt_blk))
                return parts

            def attn_qb(j, qb, fillers=()):
                """Pipelined attention for one (j, qb). The tiny tail group
                (G5, one kt) is scored FIRST so its exp is long done when its
                PV runs last (it carries the accumulation stop)."""
                ctx_ab = alloc_ctx()
                fill = list(fillers)
                prev = None
                for gi, (k0, gn) in enumerate(GROUPS[:5]):
                    w = emit_scores(j, qb, k0, gn)
                    if prev is not None:
                        emit_pv(j, ctx_ab, prev)
                    if fill:
                        fill.pop(0)()
                    prev = w
                w_last = emit_scores(j, qb, *GROUPS[5])
                emit_pv(j, ctx_ab, prev)
                for f in fill:
                    f()
                emit_pv(j, ctx_ab, w_last)
                emit_attn_epilogue(j, qb, ctx_ab)

            tail_oproj = []  # last q-bank's oproj, deferred into the next rep
            for _rep in range(reps):
                xt_blocks.clear()
                # (j0, qb0) startup: emit each projection as soon as its own
                # DMAs are in the cumulative count; scores(G_i) depend on K/Q
                # of the lb covering its kt range.
                ctx00 = alloc_ctx()
                ensure_w("k")
                xt0 = alloc_xt_block(0)
                emit_qk_group(kt_t[(0, 0)], wk_t, wk1, 0, xt0)
                # previous rep's tail oproj runs here, under this rep's
                # DMA-bound startup window (steady-state reps only)
                for f in tail_oproj:
                    f()
                tail_oproj = []
                ensure_w("q")
                emit_qk_group(qt_t[(0, 0)], wq_t, wq1, 0, xt0)
                w0 = emit_scores(0, 0, *GROUPS[0])
                ensure_w("v")
                for lt in range(4):
                    emit_v_group(lt, xt0)
                for p in lb_parts(0, 1, True):
                    p()
                w1 = emit_scores(0, 0, *GROUPS[1])
                emit_pv(0, ctx00, w0)
                for p in lb_parts(0, 2, True):
                    p()
                w2 = emit_scores(0, 0, *GROUPS[2])
                emit_pv(0, ctx00, w1)
                w3 = emit_scores(0, 0, *GROUPS[3])
                emit_pv(0, ctx00, w2)
                for p in lb_parts(0, 3, True):
                    p()
                w4 = emit_scores(0, 0, *GROUPS[4])
                emit_pv(0, ctx00, w3)
                w5 = emit_scores(0, 0, *GROUPS[5])
                emit_pv(0, ctx00, w4)
                emit_pv(0, ctx00, w5)
                emit_attn_epilogue(0, 0, ctx00)

                # j=1 projections ride as fillers under the attention stream.
                # K(1,3) must land before attn_qb(1,0) (its G5 scores read it);
                # Q(1,3) is only read by attn_qb(1,3) and fills (1,0).
                k13, q13 = lb_parts(1, 3, False)
                attn_qb(0, 1, fillers=lb_parts(1, 0, False) + [lambda: ensure_w("o")])
                attn_qb(0, 2, fillers=lb_parts(1, 1, False))
                attn_qb(0, 3, fillers=lb_parts(1, 2, False) + [k13])
                attn_qb(1, 0, fillers=[q13])
                for qb in range(1, QB):
                    attn_qb(1, qb,
                            fillers=[lambda qi=qi: emit_oproj_chunk(qb - 1, qi)
                                     for qi in range(4)])
                tail_oproj = [lambda qi=qi: emit_oproj_chunk(QB - 1, qi)
                              for qi in range(4)]
            for f in tail_oproj:
                f()

    if split:
        _split_matmul_waits(nc)
    return nc


def _split_matmul_waits(nc):
    """Walrus allows at most 2 sync commands (waits+updates) per PE matmul.

    Move surplus waits onto same-engine NOPs inserted immediately before
    the instruction (engine streams are in-order, so semantics hold).
    """
    import concourse.mybir as mybir

    SPLIT_KINDS = {
        "InstMatmult", "InstDMACopy", "InstActivation", "InstTensorCopy",
        "InstTensorTensor", "InstMemset", "InstReciprocal", "InstTensorReduce",
        "InstTensorScalar", "InstTensorScalarPtr", "InstCopy", "InstDrain",
    }
    nop_id = 0
    for fn in nc.m.functions:
        for bb in fn.blocks:
            insts = bb.instructions
            out = []
            changed = False
            for inst in insts:
                si = getattr(inst, "sync_info", None)
                kind = type(inst).__name__
                budget_total = 1 if kind in ("InstDrain", "InstNoOp") else 2
                if (
                    kind in SPLIT_KINDS
                    and si is not None
                    and si.on_wait
                    and len(si.on_wait) + len(si.on_update or []) > budget_total
                ):
                    budget = budget_total - len(si.on_update or [])
                    keep = si.on_wait[-budget:] if budget > 0 else []
                    surplus = si.on_wait[: len(si.on_wait) - len(keep)]
                    for w in surplus:
                        nop = mybir.InstNoOp(
                            name=f"I-waitnop{nop_id}",
                            engine=inst.engine,
                            ins=[],
                            outs=[],
                            sync_info=mybir.SyncInfo(on_wait=[w], on_update=[]),
                        )
                        nop_id += 1
                        out.append(nop)
                    inst.sync_info = mybir.SyncInfo(
                        on_wait=keep, on_update=si.on_update
                    )
                    changed = True
                out.append(inst)
            if changed:
                bb.instructions = out
    return nc


def _get_program(split=True):
    global _PROG, _PROG_UNSPLIT
    if split:
        if _PROG is None:
            _PROG = _build_program(split=True)
        return _PROG
    if _PROG_UNSPLIT is None:
        _PROG_UNSPLIT = _build_program(split=False)
    return _PROG_UNSPLIT


def _make_in_maps(x, Wq, bq, Wk, bk, Wv, bv, Wo, bo):
    import ml_dtypes

    f = np.float32
    bf = ml_dtypes.bfloat16
    a = lambda v: np.ascontiguousarray(v, dtype=f)
    ab = lambda v: np.ascontiguousarray(np.asarray(v, dtype=f), dtype=bf)
    in_maps = []
    for c in range(N_CORES):
        b, g = c // 4, c % 4
        s = slice(g * M_LOC, (g + 1) * M_LOC)
        in_maps.append({
            "xta": ab(x[b].T),
            "wqa": ab(Wq[s, :].T), "wqb": a(bq[s].reshape(2, 128).T),
            "wka": ab(Wk[s, :].T), "wkb": a(bk[s].reshape(2, 128).T),
            "wva": ab(Wv[s, :].T), "wvb": a(np.broadcast_to(bv[s][None, :], (128, M_LOC))),
            "woa": ab(Wo[:, s].T), "wob": a(np.broadcast_to(0.25 * bo[None, :], (128, D))),
        })
    return in_maps


def benchmark(reps=15, calls_a=2, calls_b=10, trials=10):
    """Estimate device time per kernel execution.

    One program variant with the per-core computation repeated `reps`
    times inside the NEFF. Timing slope is taken across PIPELINED call
    counts (async dispatch, single block at the end), which cancels the
    multi-ms and highly variable axon per-call overhead that broke the
    two-variant slope. Returns ns per kernel execution.
    """
    import time

    import jax
    from jax.sharding import Mesh, NamedSharding, PartitionSpec
    from jax.experimental.shard_map import shard_map

    import concourse.mybir as mybir
    from concourse import bass2jax

    bass2jax.install_neuronx_cc_hook()

    rng = np.random.default_rng(0)
    fake = dict(
        x=rng.standard_normal((2, L, D)).astype(np.float32),
        Wq=(rng.standard_normal((D, D)) * 0.03).astype(np.float32),
        bq=(rng.standard_normal(D) * 0.01).astype(np.float32),
        Wk=(rng.standard_normal((D, D)) * 0.03).astype(np.float32),
        bk=(rng.standard_normal(D) * 0.01).astype(np.float32),
        Wv=(rng.standard_normal((D, D)) * 0.03).astype(np.float32),
        bv=(rng.standard_normal(D) * 0.01).astype(np.float32),
        Wo=(rng.standard_normal((D, D)) * 0.03).astype(np.float32),
        bo=(rng.standard_normal(D) * 0.01).astype(np.float32),
    )
    in_maps = _make_in_maps(**fake)

    def run_variant(reps):
        nc = _build_program(split=True, reps=reps)

        in_names, out_info = [], []
        pn = nc.partition_id_tensor.name if nc.partition_id_tensor else None
        for alloc in nc.m.functions[0].allocations:
            if not isinstance(alloc, mybir.MemoryLocationSet):
                continue
            name = alloc.memorylocations[0].name
            if alloc.kind == "ExternalInput" and name != pn:
                in_names.append(name)
            elif alloc.kind == "ExternalOutput":
                out_info.append(
                    (name, tuple(alloc.tensor_shape), mybir.dt.np(alloc.dtype))
                )
        out_names = [n for n, _, _ in out_info]
        out_avals = [jax.core.ShapedArray(s, d) for _, s, d in out_info]
        all_in = in_names + out_names + ([pn] if pn else [])

        def _body(*args):
            operands = list(args)
            if pn is not None:
                operands.append(bass2jax.partition_id_tensor())
            return tuple(bass2jax._bass_exec_p.bind(
                *operands,
                out_avals=tuple(out_avals),
                in_names=tuple(all_in),
                out_names=tuple(out_names),
                lowering_input_output_aliases=(),
                sim_require_finite=True,
                sim_require_nnan=True,
                nc=nc,
            ))

        devices = jax.devices()[:N_CORES]
        mesh = Mesh(np.asarray(devices), ("core",))
        nio = len(in_names) + len(out_names)
        f = jax.jit(shard_map(
            _body, mesh=mesh,
            in_specs=(PartitionSpec("core"),) * nio,
            out_specs=(PartitionSpec("core"),) * len(out_names),
            check_rep=False,
        ), keep_unused=True)
        sh = NamedSharding(mesh, PartitionSpec("core"))
        args = [
            jax.device_put(
                np.concatenate(
                    [np.asarray(in_maps[c][n]) for c in range(N_CORES)], axis=0
                ),
                sh,
            )
            for n in in_names
        ] + [
            jax.device_put(np.zeros((N_CORES * s[0], *s[1:]), d), sh)
            for _, s, d in out_info
        ]
        jax.block_until_ready(f(*args))

        def timed(ncalls):
            r = None
            t0 = time.perf_counter()
            for _ in range(ncalls):
                r = f(*args)
            jax.block_until_ready(r)
            return time.perf_counter() - t0

        timed(2)  # warm the dispatch path
        slopes = []
        for _ in range(trials):
            ta = timed(calls_a)
            tb = timed(calls_b)
            slopes.append((tb - ta) / (calls_b - calls_a))
        if os.environ.get("BENCH_VERBOSE"):
            print("bench slopes/exec us:",
                  [f"{s / reps * 1e6:.1f}" for s in slopes])
        return min(slopes) / reps * 1e9

    return run_variant(reps)


def kernel(x, Wq, bq, Wk, bk, Wv, bv, Wo, bo):
    global LAST_RESULTS
    x = np.asarray(x, dtype=np.float32)
    nc = _get_program()
    in_maps = _make_in_maps(
        x, np.asarray(Wq), np.asarray(bq), np.asarray(Wk), np.asarray(bk),
        np.asarray(Wv), np.asarray(bv), np.asarray(Wo), np.asarray(bo),
    )

    if os.environ.get("BASS_KERNEL_SIM"):
        from concourse.bass_interp import CoreSim

        nc = _get_program(split=False)
        results = []
        for c in range(int(os.environ.get("BASS_KERNEL_SIM_CORES", N_CORES))):
            sim = CoreSim(nc)
            for name, val in in_maps[c].items():
                sim.tensor(name)[:] = val
            sim.simulate()
            results.append({"outp": np.array(sim.tensor("outp"))})
    else:
        from concourse import bass2jax

        results = bass2jax.run_bass_via_pjrt(nc, in_maps, n_cores=N_CORES)

    B = x.shape[0]
    out = np.stack([
        np.sum([results[4 * b + g]["outp"] for g in range(4)], axis=0)
        for b in range(B)
    ]).astype(np.float32)
    return out



# revision 66
# speedup vs baseline: 1.0036x; 1.0036x over previous
"""Multi-head self-attention (B=2, L=2048, D=1024, H=16, hd=64) on 8 trn2 cores.

Sharding: core c = 4*b + g  (b = batch, g = head-group of 4 heads).
Each core computes Q/K/V projections for its 256 hidden dims (4 heads),
attention for those heads, and a partial output projection
(ctx_g @ Wo[:, g-slice].T + bo/4).  Host sums the 4 partials per batch.

Device algorithm (per core):
  - Inputs arrive pre-transposed from host: xT [1024, 2048] (d-major),
    WqT/WkT/WvT [1024, 256], WoT [256, 1024], all bf16 (halves the input
    DMA). Q^T/K^T are kept in float32r (full fp32 bits; the PE streams
    1 cycle/row vs 4 for float32) so the scores feeding exp stay sharp;
    P/V/ctx are bf16.
  - Weight DMAs are issued lazily right before their first consumer
    (consumers wait on the cumulative DMA count, so unrelated earlier
    DMAs delay them), and PE warmup matmuls run under the input-DMA
    window to hold the HAM clock gate open.
  - QT/KT = W.T-projections in [m, L] layout (m on partitions) so that
    S^T = K Q^T comes straight out of the PE per (k-tile, q-bank) with
    k on partitions and q on the free dim. Head pairs ride PE row groups
    0-63/64-127 (hh-interleaved emission for row-group concurrency).
    Q/K biases ride the DVE evacuation as tensor_scalar adds.
  - P^T = exp(S^T/8) on the scalar engine (PSUM -> bf16 SBUF), grouped 3
    k-tiles per activation op to amortize the ~350-cycle op overhead.
  - ctx^T = [V | 1].T-weighted PV matmul (bf16): the appended ones column
    makes PSUM row 64 the softmax denominator for each q.
  - Software pipelining: PV lags scores+exp by one group; the tiny tail
    group (kt 15) is scored first per (j,qb); projections and output-
    projection chunks slot between groups as PE filler while exp runs.
  - Normalization: recip(denoms) -> rank-1 matmul broadcast -> multiply
    during PSUM evacuation (DVE).
  - Output projection + bias via DVE add during evacuation (V bias too).
"""

import os
import sys

import numpy as np

for _p in ("/opt/trn_rl_repo", "/root/.axon_site/_ro/trn_rl_repo"):
    if os.path.isdir(_p) and _p not in sys.path:
        sys.path.insert(0, _p)

L = 2048
D = 1024
HD = 64
H_LOC = 4  # heads per core
M_LOC = H_LOC * HD  # 256 hidden dims per core
N_CORES = 8
KT_TILES = L // 128  # 16 k tiles
QB = L // 512  # 4 q banks
DT_TILES = D // 128  # 8 contraction tiles for projections

_PROG = None
_PROG_UNSPLIT = None
LAST_RESULTS = None  # BassKernelResults of the most recent HW run


def _build_program(split=True, reps=1):
    import concourse.bass as bass
    import concourse.mybir as mybir
    import concourse.tile as tile

    fp32 = mybir.dt.float32
    f32r = mybir.dt.float32r
    bf16 = mybir.dt.bfloat16
    Exp = mybir.ActivationFunctionType.Exp

    # Matmul dtypes: float32r (same fp32 bit layout, 1 PE cycle/row vs 4 for
    # float32 at moving free dim >= 256) for the precision-sensitive Q/K path
    # (scores feed exp); bf16 for x/weights/P/V/ctx where rounding is benign.
    # Every fp32r matmul operand must be PRODUCED as fp32r (DMA, DVE copy,
    # scalar activation all qualify) or the BIR verifier rejects the program.
    nc = bass.Bass()

    xta = nc.dram_tensor("xta", [D, L], bf16, kind="ExternalInput")
    wqa = nc.dram_tensor("wqa", [D, M_LOC], bf16, kind="ExternalInput")
    wka = nc.dram_tensor("wka", [D, M_LOC], bf16, kind="ExternalInput")
    wva = nc.dram_tensor("wva", [D, M_LOC], bf16, kind="ExternalInput")
    wqb = nc.dram_tensor("wqb", [128, 2], f32r, kind="ExternalInput")
    wkb = nc.dram_tensor("wkb", [128, 2], f32r, kind="ExternalInput")
    wvb = nc.dram_tensor("wvb", [128, M_LOC], f32r, kind="ExternalInput")
    woa = nc.dram_tensor("woa", [M_LOC, D], bf16, kind="ExternalInput")
    wob = nc.dram_tensor("wob", [128, D], f32r, kind="ExternalInput")
    outp = nc.dram_tensor("outp", [L, D], fp32, kind="ExternalOutput")

    with nc.allow_low_precision(reason="float32r is fp32-width; rounding loss is negligible"), tile.TileContext(nc) as tc:
        with (
            tc.tile_pool(name="const", bufs=1) as cpool,
            tc.tile_pool(name="pt", bufs=2) as ptpool,
            tc.tile_pool(name="ev", bufs=2) as epool,
            tc.tile_pool(name="psum", bufs=2, space="PSUM") as psum,
        ):
            # ---- persistent SBUF tiles ----
            # weights live as one [128, dt, M_LOC] tile each: one DMA per
            # matrix (DMA dispatch serializes at ~650ns/instruction)
            wq_t = cpool.tile([128, DT_TILES, M_LOC], bf16, tag="wq", name="wq")
            wk_t = cpool.tile([128, DT_TILES, M_LOC], bf16, tag="wk", name="wk")
            wv_t = cpool.tile([128, DT_TILES, M_LOC], bf16, tag="wv", name="wv")
            wq1 = cpool.tile([128, 2], f32r, tag="wqbias", name="wqbias")
            wk1 = cpool.tile([128, 2], f32r, tag="wkbias", name="wkbias")
            wv1 = cpool.tile([128, M_LOC], f32r, tag="wvbias", name="wvbias")
            wo_t = [cpool.tile([128, D], bf16, tag=f"wo{j}", name=f"wo{j}") for j in range(2)]
            wo1 = cpool.tile([128, D], f32r, tag="wobias", name="wobias")
            qt_t = {}  # (j, lb) -> Q^T [m-tile 128, 512]
            kt_t = {}
            for j in range(2):
                for lb in range(QB):
                    qt_t[(j, lb)] = cpool.tile([128, 512], f32r, tag=f"qt{j}_{lb}", name=f"qt{j}_{lb}")
                    kt_t[(j, lb)] = cpool.tile([128, 512], f32r, tag=f"kt{j}_{lb}", name=f"kt{j}_{lb}")
            # V with appended ones column: [l-part, h, 65]
            v_t = [cpool.tile([128, H_LOC, HD + 1], bf16, tag=f"v{lt}", name=f"v{lt}")
                   for lt in range(KT_TILES)]
            ctxn = {}  # (j, qb) -> normalized ctx^T [128 m, 512 q]
            for j in range(2):
                for qb in range(QB):
                    ctxn[(j, qb)] = cpool.tile([128, 512], bf16, tag=f"cn{j}_{qb}", name=f"cn{j}_{qb}")
            ones = cpool.tile([128, 512], f32r, tag="ones", name="ones")[0:1, :]
            warm = cpool.tile([128, 8], fp32, tag="warm", name="warm")[0:1, :]

            # ---- constants / warmup ----
            nc.vector.memset(ones[:].bitcast(fp32), 1.0)
            for lt in range(KT_TILES):
                nc.vector.memset(v_t[lt][:, :, HD:HD + 1], 1.0)
            # trigger the exp table load early (hides under input DMA)
            nc.scalar.activation(out=warm[:], in_=ones[0:1, 0:8].bitcast(fp32), func=Exp)
            # PE warmup under the input-DMA window: keeps the HAM activity
            # monitor busy so the real matmuls start at 2.4 GHz instead of
            # paying the cold 1.2 GHz window. Reuses the ctx PSUM tag (no
            # extra bank) and is done long before the first PV needs it.
            wps = cpool_warm = psum.tile([1, 512], fp32, tag="ctx", name="warmps")
            for _ in range(16):
                nc.tensor.matmul(
                    wps[:], ones[0:1, 0:1], ones[0:1, :], start=True, stop=True
                )

            # ---- emission helpers ----
            xt_blocks = {}

            xta_r = xta[:].rearrange("(dt p) l -> p dt l", p=128)

            def alloc_xt_block(lb):
                """One DMA brings the whole 512-wide L chunk of x^T as a
                [128, dt, 512] tile (kept resident so the j=1 projection
                pass reuses it)."""
                if lb in xt_blocks:
                    return xt_blocks[lb]
                t = ptpool.tile([128, DT_TILES, 512], bf16, tag="xt", name="xtb",
                                bufs=4)
                nc.sync.dma_start(t[:], xta_r[:, :, lb * 512:(lb + 1) * 512])
                xt_blocks[lb] = t
                return t

            # ---- weight DMAs, issued lazily: consumers wait on the
            # CUMULATIVE dma count, so a DMA emitted before an instruction
            # delays it even if unrelated. Each weight group is issued right
            # before its first consumer. ----
            wv1_t, wo1_t = wv1, wo1
            wv1, wo1 = wv1[0:1, :], wo1[0:1, :]
            _w_issued = set()

            def ensure_w(which):
                if which in _w_issued:
                    return
                _w_issued.add(which)
                if which == "k":
                    nc.sync.dma_start(wk1[:], wkb[:])
                    nc.sync.dma_start(
                        wk_t[:], wka[:].rearrange("(dt p) m -> p dt m", p=128))
                elif which == "q":
                    nc.sync.dma_start(wq1[:], wqb[:])
                    nc.sync.dma_start(
                        wq_t[:], wqa[:].rearrange("(dt p) m -> p dt m", p=128))
                elif which == "v":
                    nc.sync.dma_start(wv1_t[:], wvb[:])
                    nc.sync.dma_start(
                        wv_t[:], wva[:].rearrange("(dt p) m -> p dt m", p=128))
                elif which == "o":
                    nc.sync.dma_start(wo1_t[:], wob[:])
                    for j in range(2):
                        nc.sync.dma_start(wo_t[j][:], woa[j * 128:(j + 1) * 128, :])

            def emit_qk_group(dst, w_tiles, w1, j, xt_blk):
                """dst[m, l] = sum_d W^T[d, m] x^T[d, l] + b[m]  (one q/k bank).

                The bias (constant along l, per-partition in m) rides the DVE
                evacuation as a tensor_scalar add instead of a rank-1 matmul.
                """
                ps = psum.tile([128, 512], fp32, tag="st", name="st")
                for dt in range(DT_TILES):
                    nc.tensor.matmul(
                        ps[:],
                        (w_tiles[:, dt, j * 128:(j + 1) * 128]),
                        (xt_blk[:, dt, :]),
                        start=(dt == 0),
                        stop=(dt == DT_TILES - 1),
                    )
                nc.vector.tensor_scalar_add(
                    out=dst[:], in0=ps[:], scalar1=w1[:, j:j + 1].bitcast(fp32)
                )

            def emit_v_group(lt, xt_blk):
                """v_t[lt][l, h, d] = sum_d' x^T[d', l] Wv^T[d', (h,d)] + bv."""
                li = lt % 4
                ps = psum.tile([128, M_LOC], fp32, tag="st", name="st")
                for dt in range(DT_TILES):
                    nc.tensor.matmul(
                        ps[:],
                        (xt_blk[:, dt, li * 128:(li + 1) * 128]),
                        (wv_t[:, dt, :]),
                        start=(dt == 0),
                        stop=(dt == DT_TILES - 1),
                    )
                # bias rides the DVE evacuation (wvb rows are all identical)
                nc.vector.tensor_add(
                    out=v_t[lt][:, :, 0:HD],
                    in0=ps.rearrange("p (h d) -> p h d", d=HD),
                    in1=wv1_t[:].bitcast(fp32).rearrange("p (h d) -> p h d", d=HD),
                )

            # kt-groups per (j, qb): sizes 3,3,3,3,3,1 (st slot = 3 banks)
            GROUPS = [(0, 3), (3, 3), (6, 3), (9, 3), (12, 3), (15, 1)]

            def emit_scores(j, qb, k0, gn):
                """S^T -> exp for kt in [k0, k0+gn); returns the PV work item."""
                sts, pts = [], []
                for hh in range(2):
                    sts.append(psum.tile([128, 3, 512], fp32, tag="st", name="st"))
                # hh-interleaved emission: consecutive matmuls target disjoint
                # PE row groups (rows 0-63 vs 64-127), so on HW each can start
                # ~4ns after the previous one -> the head pair runs concurrent.
                for u in range(gn):
                    kt = k0 + u
                    for hh in range(2):
                        r0, r1 = hh * HD, (hh + 1) * HD
                        nc.tensor.matmul(
                            sts[hh][:, u, :],
                            (kt_t[(j, kt // 4)][r0:r1, (kt % 4) * 128:(kt % 4 + 1) * 128]),
                            (qt_t[(j, qb)][r0:r1, :]),
                            start=True,
                            stop=True,
                        )
                for hh in range(2):
                    # bufs=3: G5 is scored first and its pts stay live across
                    # the whole qb while the main pipeline double-buffers
                    pt = ptpool.tile([128, 3, 512], bf16, tag="pt", name="pt", bufs=6)
                    pts.append(pt)
                    if os.environ.get("KABL_NOEXP"):
                        nc.vector.tensor_copy(out=pt[:, 0:gn, :], in_=sts[hh][:, 0:gn, :])
                    else:
                        nc.scalar.activation(
                            out=pt[:, 0:gn, :], in_=sts[hh][:, 0:gn, :],
                            func=Exp, scale=0.125,
                        )
                return (pts, k0, gn)

            def emit_pv(j, ctx_ab, work):
                pts, k0, gn = work
                for u in range(gn):
                    kt = k0 + u
                    for hh in range(2):
                        nc.tensor.matmul(
                            ctx_ab[hh][:],
                            (v_t[kt][:, 2 * j + hh, :]),
                            (pts[hh][:, u, :]),
                            start=(kt == 0),
                            stop=(kt == KT_TILES - 1),
                        )

            def emit_attn_epilogue(j, qb, ctx_ab):
                for hh in range(2):
                    rec = epool.tile([1, 512], f32r, tag="rec", name="rec")
                    nc.vector.reciprocal(rec[:], ctx_ab[hh][HD:HD + 1, :])
                    rp = psum.tile([HD, 512], fp32, tag="st", name="st")
                    nc.tensor.matmul(
                        rp[:], (ones[0:1, 0:HD]), (rec[:]), start=True, stop=True
                    )
                    # NB: one DVE op must not read two PSUM operands (single
                    # PSUM port) — stage rp through SBUF.
                    rsb = epool.tile([HD, 512], fp32, tag="rsb", name="rsb")
                    nc.vector.tensor_copy(out=rsb[:], in_=rp[:])
                    nc.vector.tensor_mul(
                        out=ctxn[(j, qb)][hh * HD:(hh + 1) * HD, :],
                        in0=ctx_ab[hh][0:HD, :],
                        in1=rsb[:],
                    )

            def emit_oproj_chunk(qb, qi):
                qt = qb * 4 + qi
                ot = epool.tile([128, 2, 512], fp32, tag="ot", name="ot")
                for nb in range(2):
                    ps = psum.tile([128, 512], fp32, tag="st", name="st")
                    for j in range(2):
                        nc.tensor.matmul(
                            ps[:],
                            (ctxn[(j, qb)][:, qi * 128:(qi + 1) * 128]),
                            (wo_t[j][:, nb * 512:(nb + 1) * 512]),
                            start=(j == 0),
                            stop=(j == 1),
                        )
                    # bias (0.25*bo, rows identical) rides the DVE evacuation
                    nc.vector.tensor_add(
                        out=ot[:, nb, :], in0=ps[:],
                        in1=wo1_t[:, nb * 512:(nb + 1) * 512].bitcast(fp32),
                    )
                # one [128, 1024] store per chunk instead of two halves
                if not os.environ.get("KABL_NOOUT"):
                    nc.sync.dma_start(
                        outp[qt * 128:(qt + 1) * 128, :],
                        ot.rearrange("p a b -> p (a b)"),
                    )

            # ---- emission schedule ----
            # Software-pipelined: PV lags scores+exp by one kt-group so the PE
            # always has independent work (next scores, projections, oproj
            # chunks) in its stream while the scalar engine runs exp.
            def alloc_ctx():
                # the first PV matmul (kt==0) opens the accumulation group
                # with start=True; surplus waits are NOP-split by
                # _split_matmul_waits.
                return [psum.tile([HD + 1, 512], fp32, tag="ctx", name="ctx")
                        for _ in range(2)]

            def lb_parts(j, lb, with_v):
                """Projection work for one L-chunk as filler thunks."""
                xt_blk = alloc_xt_block(lb)
                parts = [
                    lambda: emit_qk_group(kt_t[(j, lb)], wk_t, wk1, j, xt_blk),
                    lambda: emit_qk_group(qt_t[(j, lb)], wq_t, wq1, j, xt_blk),
                ]
                if with_v:
                    for lt in range(lb * 4, lb * 4 + 4):
                        parts.append(lambda lt=lt: emit_v_group(lt, xt_blk))
                return parts

            def attn_qb(j, qb, fillers=()):
                """Pipelined attention for one (j, qb). The tiny tail group
                (G5, one kt) is scored FIRST so its exp is long done when its
                PV runs last (it carries the accumulation stop)."""
                ctx_ab = alloc_ctx()
                fill = list(fillers)
                w_last = emit_scores(j, qb, *GROUPS[5])
                prev = None
                for gi, (k0, gn) in enumerate(GROUPS[:5]):
                    w = emit_scores(j, qb, k0, gn)
                    if prev is not None:
                        emit_pv(j, ctx_ab, prev)
                    if fill:
                        fill.pop(0)()
                    prev = w
                emit_pv(j, ctx_ab, prev)
                for f in fill:
                    f()
                emit_pv(j, ctx_ab, w_last)
                emit_attn_epilogue(j, qb, ctx_ab)

            tail_oproj = []  # last q-bank's oproj, deferred into the next rep
            for _rep in range(reps):
                xt_blocks.clear()
                # (j0, qb0) startup: emit each projection as soon as its own
                # DMAs are in the cumulative count; scores(G_i) depend on K/Q
                # of the lb covering its kt range.
                ctx00 = alloc_ctx()
                ensure_w("k")
                xt0 = alloc_xt_block(0)
                emit_qk_group(kt_t[(0, 0)], wk_t, wk1, 0, xt0)
                # previous rep's tail oproj runs here, under this rep's
                # DMA-bound startup window (steady-state reps only)
                for f in tail_oproj:
                    f()
                tail_oproj = []
                ensure_w("q")
                emit_qk_group(qt_t[(0, 0)], wq_t, wq1, 0, xt0)
                w0 = emit_scores(0, 0, *GROUPS[0])
                ensure_w("v")
                for lt in range(4):
                    emit_v_group(lt, xt0)
                for p in lb_parts(0, 1, True):
                    p()
                w1 = emit_scores(0, 0, *GROUPS[1])
                emit_pv(0, ctx00, w0)
                for p in lb_parts(0, 2, True):
                    p()
                w2 = emit_scores(0, 0, *GROUPS[2])
                emit_pv(0, ctx00, w1)
                w3 = emit_scores(0, 0, *GROUPS[3])
                emit_pv(0, ctx00, w2)
                for p in lb_parts(0, 3, True):
                    p()
                w4 = emit_scores(0, 0, *GROUPS[4])
                emit_pv(0, ctx00, w3)
                w5 = emit_scores(0, 0, *GROUPS[5])
                emit_pv(0, ctx00, w4)
                emit_pv(0, ctx00, w5)
                emit_attn_epilogue(0, 0, ctx00)

                # j=1 projections ride as fillers under the attention stream.
                # K(1,3) must land before attn_qb(1,0) (its G5 scores read it);
                # Q(1,3) is only read by attn_qb(1,3) and fills (1,0).
                k13, q13 = lb_parts(1, 3, False)
                attn_qb(0, 1, fillers=lb_parts(1, 0, False) + [lambda: ensure_w("o")])
                attn_qb(0, 2, fillers=lb_parts(1, 1, False))
                attn_qb(0, 3, fillers=lb_parts(1, 2, False) + [k13])
                attn_qb(1, 0, fillers=[q13])
                for qb in range(1, QB):
                    attn_qb(1, qb,
                            fillers=[lambda qi=qi: emit_oproj_chunk(qb - 1, qi)
                                     for qi in range(4)])
                tail_oproj = [lambda qi=qi: emit_oproj_chunk(QB - 1, qi)
                              for qi in range(4)]
            for f in tail_oproj:
                f()

    if split:
        _split_matmul_waits(nc)
    return nc


def _split_matmul_waits(nc):
    """Walrus allows at most 2 sync commands (waits+updates) per PE matmul.

    Move surplus waits onto same-engine NOPs inserted immediately before
    the instruction (engine streams are in-order, so semantics hold).
    """
    import concourse.mybir as mybir

    SPLIT_KINDS = {
        "InstMatmult", "InstDMACopy", "InstActivation", "InstTensorCopy",
        "InstTensorTensor", "InstMemset", "InstReciprocal", "InstTensorReduce",
        "InstTensorScalar", "InstTensorScalarPtr", "InstCopy", "InstDrain",
    }
    nop_id = 0
    for fn in nc.m.functions:
        for bb in fn.blocks:
            insts = bb.instructions
            out = []
            changed = False
            for inst in insts:
                si = getattr(inst, "sync_info", None)
                kind = type(inst).__name__
                budget_total = 1 if kind in ("InstDrain", "InstNoOp") else 2
                if (
                    kind in SPLIT_KINDS
                    and si is not None
                    and si.on_wait
                    and len(si.on_wait) + len(si.on_update or []) > budget_total
                ):
                    budget = budget_total - len(si.on_update or [])
                    keep = si.on_wait[-budget:] if budget > 0 else []
                    surplus = si.on_wait[: len(si.on_wait) - len(keep)]
                    for w in surplus:
                        nop = mybir.InstNoOp(
                            name=f"I-waitnop{nop_id}",
                            engine=inst.engine,
                            ins=[],
                            outs=[],
                            sync_info=mybir.SyncInfo(on_wait=[w], on_update=[]),
                        )
                        nop_id += 1
                        out.append(nop)
                    inst.sync_info = mybir.SyncInfo(
                        on_wait=keep, on_update=si.on_update
                    )
                    changed = True
                out.append(inst)
            if changed:
                bb.instructions = out
    return nc


def _get_program(split=True):
    global _PROG, _PROG_UNSPLIT
    if split:
        if _PROG is None:
            _PROG = _build_program(split=True)
        return _PROG
    if _PROG_UNSPLIT is None:
        _PROG_UNSPLIT = _build_program(split=False)
    return _PROG_UNSPLIT


def _make_in_maps(x, Wq, bq, Wk, bk, Wv, bv, Wo, bo):
    import ml_dtypes

    f = np.float32
    bf = ml_dtypes.bfloat16
    a = lambda v: np.ascontiguousarray(v, dtype=f)
    ab = lambda v: np.ascontiguousarray(np.asarray(v, dtype=f), dtype=bf)
    in_maps = []
    for c in range(N_CORES):
        b, g = c // 4, c % 4
        s = slice(g * M_LOC, (g + 1) * M_LOC)
        in_maps.append({
            "xta": ab(x[b].T),
            "wqa": ab(Wq[s, :].T), "wqb": a(bq[s].reshape(2, 128).T),
            "wka": ab(Wk[s, :].T), "wkb": a(bk[s].reshape(2, 128).T),
            "wva": ab(Wv[s, :].T), "wvb": a(np.broadcast_to(bv[s][None, :], (128, M_LOC))),
            "woa": ab(Wo[:, s].T), "wob": a(np.broadcast_to(0.25 * bo[None, :], (128, D))),
        })
    return in_maps


def benchmark(reps=15, calls_a=2, calls_b=10, trials=10):
    """Estimate device time per kernel execution.

    One program variant with the per-core computation repeated `reps`
    times inside the NEFF. Timing slope is taken across PIPELINED call
    counts (async dispatch, single block at the end), which cancels the
    multi-ms and highly variable axon per-call overhead that broke the
    two-variant slope. Returns ns per kernel execution.
    """
    import time

    import jax
    from jax.sharding import Mesh, NamedSharding, PartitionSpec
    from jax.experimental.shard_map import shard_map

    import concourse.mybir as mybir
    from concourse import bass2jax

    bass2jax.install_neuronx_cc_hook()

    rng = np.random.default_rng(0)
    fake = dict(
        x=rng.standard_normal((2, L, D)).astype(np.float32),
        Wq=(rng.standard_normal((D, D)) * 0.03).astype(np.float32),
        bq=(rng.standard_normal(D) * 0.01).astype(np.float32),
        Wk=(rng.standard_normal((D, D)) * 0.03).astype(np.float32),
        bk=(rng.standard_normal(D) * 0.01).astype(np.float32),
        Wv=(rng.standard_normal((D, D)) * 0.03).astype(np.float32),
        bv=(rng.standard_normal(D) * 0.01).astype(np.float32),
        Wo=(rng.standard_normal((D, D)) * 0.03).astype(np.float32),
        bo=(rng.standard_normal(D) * 0.01).astype(np.float32),
    )
    in_maps = _make_in_maps(**fake)

    def run_variant(reps):
        nc = _build_program(split=True, reps=reps)

        in_names, out_info = [], []
        pn = nc.partition_id_tensor.name if nc.partition_id_tensor else None
        for alloc in nc.m.functions[0].allocations:
            if not isinstance(alloc, mybir.MemoryLocationSet):
                continue
            name = alloc.memorylocations[0].name
            if alloc.kind == "ExternalInput" and name != pn:
                in_names.append(name)
            elif alloc.kind == "ExternalOutput":
                out_info.append(
                    (name, tuple(alloc.tensor_shape), mybir.dt.np(alloc.dtype))
                )
        out_names = [n for n, _, _ in out_info]
        out_avals = [jax.core.ShapedArray(s, d) for _, s, d in out_info]
        all_in = in_names + out_names + ([pn] if pn else [])

        def _body(*args):
            operands = list(args)
            if pn is not None:
                operands.append(bass2jax.partition_id_tensor())
            return tuple(bass2jax._bass_exec_p.bind(
                *operands,
                out_avals=tuple(out_avals),
                in_names=tuple(all_in),
                out_names=tuple(out_names),
                lowering_input_output_aliases=(),
                sim_require_finite=True,
                sim_require_nnan=True,
                nc=nc,
            ))

        devices = jax.devices()[:N_CORES]
        mesh = Mesh(np.asarray(devices), ("core",))
        nio = len(in_names) + len(out_names)
        f = jax.jit(shard_map(
            _body, mesh=mesh,
            in_specs=(PartitionSpec("core"),) * nio,
            out_specs=(PartitionSpec("core"),) * len(out_names),
            check_rep=False,
        ), keep_unused=True)
        sh = NamedSharding(mesh, PartitionSpec("core"))
        args = [
            jax.device_put(
                np.concatenate(
                    [np.asarray(in_maps[c][n]) for c in range(N_CORES)], axis=0
                ),
                sh,
            )
            for n in in_names
        ] + [
            jax.device_put(np.zeros((N_CORES * s[0], *s[1:]), d), sh)
            for _, s, d in out_info
        ]
        jax.block_until_ready(f(*args))

        def timed(ncalls):
            r = None
            t0 = time.perf_counter()
            for _ in range(ncalls):
                r = f(*args)
            jax.block_until_ready(r)
            return time.perf_counter() - t0

        timed(2)  # warm the dispatch path
        slopes = []
        for _ in range(trials):
            ta = timed(calls_a)
            tb = timed(calls_b)
            slopes.append((tb - ta) / (calls_b - calls_a))
        if os.environ.get("BENCH_VERBOSE"):
            print("bench slopes/exec us:",
                  [f"{s / reps * 1e6:.1f}" for s in slopes])
        return min(slopes) / reps * 1e9

    return run_variant(reps)


def kernel(x, Wq, bq, Wk, bk, Wv, bv, Wo, bo):
    global LAST_RESULTS
    x = np.asarray(x, dtype=np.float32)
    nc = _get_program()
    in_maps = _make_in_maps(
        x, np.asarray(Wq), np.asarray(bq), np.asarray(Wk), np.asarray(bk),
        np.asarray(Wv), np.asarray(bv), np.asarray(Wo), np.asarray(bo),
    )

    if os.environ.get("BASS_KERNEL_SIM"):
        from concourse.bass_interp import CoreSim

        nc = _get_program(split=False)
        results = []
        for c in range(int(os.environ.get("BASS_KERNEL_SIM_CORES", N_CORES))):
            sim = CoreSim(nc)
            for name, val in in_maps[c].items():
                sim.tensor(name)[:] = val
            sim.simulate()
            results.append({"outp": np.array(sim.tensor("outp"))})
    else:
        from concourse import bass2jax

        results = bass2jax.run_bass_via_pjrt(nc, in_maps, n_cores=N_CORES)

    B = x.shape[0]
    out = np.stack([
        np.sum([results[4 * b + g]["outp"] for g in range(4)], axis=0)
        for b in range(B)
    ]).astype(np.float32)
    return out



# revision 68
# speedup vs baseline: 1.1416x; 1.1374x over previous
"""Multi-head self-attention (B=2, L=2048, D=1024, H=16, hd=64) on 8 trn2 cores.

Sharding: core c = 4*b + g  (b = batch, g = head-group of 4 heads).
Each core computes Q/K/V projections for its 256 hidden dims (4 heads),
attention for those heads, and a partial output projection
(ctx_g @ Wo[:, g-slice].T + bo/4).  Host sums the 4 partials per batch.

Device algorithm (per core):
  - Inputs arrive pre-transposed from host: xT [1024, 2048] (d-major),
    WqT/WkT/WvT [1024, 256], WoT [256, 1024], all bf16 (halves the input
    DMA). Q^T/K^T are kept in float32r (full fp32 bits; the PE streams
    1 cycle/row vs 4 for float32) so the scores feeding exp stay sharp;
    P/V/ctx are bf16.
  - Weight DMAs are issued lazily right before their first consumer
    (consumers wait on the cumulative DMA count, so unrelated earlier
    DMAs delay them), and PE warmup matmuls run under the input-DMA
    window to hold the HAM clock gate open.
  - QT/KT = W.T-projections in [m, L] layout (m on partitions) so that
    S^T = K Q^T comes straight out of the PE per (k-tile, q-bank) with
    k on partitions and q on the free dim. Head pairs ride PE row groups
    0-63/64-127 (hh-interleaved emission for row-group concurrency).
    Q/K biases ride the DVE evacuation as tensor_scalar adds.
  - P^T = exp(S^T/8) on the scalar engine (PSUM -> bf16 SBUF), grouped 3
    k-tiles per activation op to amortize the ~350-cycle op overhead.
  - ctx^T = [V | 1].T-weighted PV matmul (bf16): the appended ones column
    makes PSUM row 64 the softmax denominator for each q.
  - Software pipelining: PV lags scores+exp by one group; the tiny tail
    group (kt 15) is scored first per (j,qb); projections and output-
    projection chunks slot between groups as PE filler while exp runs.
  - Normalization: recip(denoms) -> rank-1 matmul broadcast -> multiply
    during PSUM evacuation (DVE).
  - Output projection + bias via DVE add during evacuation (V bias too).
"""

import os
import sys

import numpy as np

for _p in ("/opt/trn_rl_repo", "/root/.axon_site/_ro/trn_rl_repo"):
    if os.path.isdir(_p) and _p not in sys.path:
        sys.path.insert(0, _p)

L = 2048
D = 1024
HD = 64
H_LOC = 4  # heads per core
M_LOC = H_LOC * HD  # 256 hidden dims per core
N_CORES = 8
KT_TILES = L // 128  # 16 k tiles
QB = L // 512  # 4 q banks
DT_TILES = D // 128  # 8 contraction tiles for projections

_PROG = None
_PROG_UNSPLIT = None
LAST_RESULTS = None  # BassKernelResults of the most recent HW run


def _build_program(split=True, reps=1):
    import concourse.bass as bass
    import concourse.mybir as mybir
    import concourse.tile as tile

    fp32 = mybir.dt.float32
    f32r = mybir.dt.float32r
    bf16 = mybir.dt.bfloat16
    Exp = mybir.ActivationFunctionType.Exp

    # Matmul dtypes: float32r (same fp32 bit layout, 1 PE cycle/row vs 4 for
    # float32 at moving free dim >= 256) for the precision-sensitive Q/K path
    # (scores feed exp); bf16 for x/weights/P/V/ctx where rounding is benign.
    # Every fp32r matmul operand must be PRODUCED as fp32r (DMA, DVE copy,
    # scalar activation all qualify) or the BIR verifier rejects the program.
    nc = bass.Bass()

    xta = nc.dram_tensor("xta", [D, L], bf16, kind="ExternalInput")
    wqa = nc.dram_tensor("wqa", [D, M_LOC], bf16, kind="ExternalInput")
    wka = nc.dram_tensor("wka", [D, M_LOC], bf16, kind="ExternalInput")
    wva = nc.dram_tensor("wva", [D, M_LOC], bf16, kind="ExternalInput")
    wqb = nc.dram_tensor("wqb", [128, 2], f32r, kind="ExternalInput")
    wkb = nc.dram_tensor("wkb", [128, 2], f32r, kind="ExternalInput")
    wvb = nc.dram_tensor("wvb", [128, M_LOC], f32r, kind="ExternalInput")
    woa = nc.dram_tensor("woa", [M_LOC, D], bf16, kind="ExternalInput")
    wob = nc.dram_tensor("wob", [128, D], f32r, kind="ExternalInput")
    outp = nc.dram_tensor("outp", [L, D], fp32, kind="ExternalOutput")

    with nc.allow_low_precision(reason="float32r is fp32-width; rounding loss is negligible"), tile.TileContext(nc) as tc:
        with (
            tc.tile_pool(name="const", bufs=1) as cpool,
            tc.tile_pool(name="pt", bufs=2) as ptpool,
            tc.tile_pool(name="ev", bufs=2) as epool,
            tc.tile_pool(name="psum", bufs=2, space="PSUM") as psum,
        ):
            # ---- persistent SBUF tiles ----
            # weights live as one [128, dt, M_LOC] tile each: one DMA per
            # matrix (DMA dispatch serializes at ~650ns/instruction)
            wq_t = cpool.tile([128, DT_TILES, M_LOC], bf16, tag="wq", name="wq")
            wk_t = cpool.tile([128, DT_TILES, M_LOC], bf16, tag="wk", name="wk")
            wv_t = cpool.tile([128, DT_TILES, M_LOC], bf16, tag="wv", name="wv")
            wq1 = cpool.tile([128, 2], f32r, tag="wqbias", name="wqbias")
            wk1 = cpool.tile([128, 2], f32r, tag="wkbias", name="wkbias")
            wv1 = cpool.tile([128, M_LOC], f32r, tag="wvbias", name="wvbias")
            wo_t = [cpool.tile([128, D], bf16, tag=f"wo{j}", name=f"wo{j}") for j in range(2)]
            wo1 = cpool.tile([128, D], f32r, tag="wobias", name="wobias")
            qt_t = {}  # (j, lb) -> Q^T [m-tile 128, 512]
            kt_t = {}
            for j in range(2):
                for lb in range(QB):
                    qt_t[(j, lb)] = cpool.tile([128, 512], f32r, tag=f"qt{j}_{lb}", name=f"qt{j}_{lb}")
                    kt_t[(j, lb)] = cpool.tile([128, 512], f32r, tag=f"kt{j}_{lb}", name=f"kt{j}_{lb}")
            # V with appended ones column: [l-part, h, 65]
            v_t = [cpool.tile([128, H_LOC, HD + 1], bf16, tag=f"v{lt}", name=f"v{lt}")
                   for lt in range(KT_TILES)]
            ctxn = {}  # (j, qb) -> normalized ctx^T [128 m, 512 q]
            for j in range(2):
                for qb in range(QB):
                    ctxn[(j, qb)] = cpool.tile([128, 512], bf16, tag=f"cn{j}_{qb}", name=f"cn{j}_{qb}")
            ones = cpool.tile([128, 512], f32r, tag="ones", name="ones")[0:1, :]
            warm = cpool.tile([128, 8], fp32, tag="warm", name="warm")[0:1, :]

            # ---- constants / warmup ----
            nc.vector.memset(ones[:].bitcast(fp32), 1.0)
            for lt in range(KT_TILES):
                nc.vector.memset(v_t[lt][:, :, HD:HD + 1], 1.0)
            # trigger the exp table load early (hides under input DMA)
            nc.scalar.activation(out=warm[:], in_=ones[0:1, 0:8].bitcast(fp32), func=Exp)
            # PE warmup under the input-DMA window: keeps the HAM activity
            # monitor busy so the real matmuls start at 2.4 GHz instead of
            # paying the cold 1.2 GHz window. Reuses the ctx PSUM tag (no
            # extra bank) and is done long before the first PV needs it.
            wps = cpool_warm = psum.tile([1, 512], fp32, tag="ctx", name="warmps")
            for _ in range(16):
                nc.tensor.matmul(
                    wps[:], ones[0:1, 0:1], ones[0:1, :], start=True, stop=True
                )

            # ---- emission helpers ----
            xt_blocks = {}

            xta_r = xta[:].rearrange("(dt p) l -> p dt l", p=128)

            def alloc_xt_block(lb):
                """One DMA brings the whole 512-wide L chunk of x^T as a
                [128, dt, 512] tile (kept resident so the j=1 projection
                pass reuses it)."""
                if lb in xt_blocks:
                    return xt_blocks[lb]
                t = ptpool.tile([128, DT_TILES, 512], bf16, tag="xt", name="xtb",
                                bufs=4)
                nc.sync.dma_start(t[:], xta_r[:, :, lb * 512:(lb + 1) * 512])
                xt_blocks[lb] = t
                return t

            # ---- weight DMAs, issued lazily: consumers wait on the
            # CUMULATIVE dma count, so a DMA emitted before an instruction
            # delays it even if unrelated. Each weight group is issued right
            # before its first consumer. ----
            wv1_t, wo1_t = wv1, wo1
            wv1, wo1 = wv1[0:1, :], wo1[0:1, :]
            _w_issued = set()

            def ensure_w(which):
                if which in _w_issued:
                    return
                _w_issued.add(which)
                if which == "k":
                    nc.sync.dma_start(wk1[:], wkb[:])
                    nc.sync.dma_start(
                        wk_t[:], wka[:].rearrange("(dt p) m -> p dt m", p=128))
                elif which == "q":
                    nc.sync.dma_start(wq1[:], wqb[:])
                    nc.sync.dma_start(
                        wq_t[:], wqa[:].rearrange("(dt p) m -> p dt m", p=128))
                elif which == "v":
                    nc.sync.dma_start(wv1_t[:], wvb[:])
                    nc.sync.dma_start(
                        wv_t[:], wva[:].rearrange("(dt p) m -> p dt m", p=128))
                elif which == "o":
                    nc.sync.dma_start(wo1_t[:], wob[:])
                    for j in range(2):
                        nc.sync.dma_start(wo_t[j][:], woa[j * 128:(j + 1) * 128, :])

            def emit_qk_group(dst, w_tiles, w1, j, xt_blk):
                """dst[m, l] = sum_d W^T[d, m] x^T[d, l] + b[m]  (one q/k bank).

                The bias (constant along l, per-partition in m) rides the DVE
                evacuation as a tensor_scalar add instead of a rank-1 matmul.
                """
                ps = psum.tile([128, 512], fp32, tag="st", name="st")
                for dt in range(DT_TILES):
                    nc.tensor.matmul(
                        ps[:],
                        (w_tiles[:, dt, j * 128:(j + 1) * 128]),
                        (xt_blk[:, dt, :]),
                        start=(dt == 0),
                        stop=(dt == DT_TILES - 1),
                    )
                nc.vector.tensor_scalar_add(
                    out=dst[:], in0=ps[:], scalar1=w1[:, j:j + 1].bitcast(fp32)
                )

            def emit_v_group(lt, xt_blk):
                """v_t[lt][l, h, d] = sum_d' x^T[d', l] Wv^T[d', (h,d)] + bv."""
                li = lt % 4
                ps = psum.tile([128, M_LOC], fp32, tag="st", name="st")
                for dt in range(DT_TILES):
                    nc.tensor.matmul(
                        ps[:],
                        (xt_blk[:, dt, li * 128:(li + 1) * 128]),
                        (wv_t[:, dt, :]),
                        start=(dt == 0),
                        stop=(dt == DT_TILES - 1),
                    )
                # bias rides the DVE evacuation (wvb rows are all identical)
                nc.vector.tensor_add(
                    out=v_t[lt][:, :, 0:HD],
                    in0=ps.rearrange("p (h d) -> p h d", d=HD),
                    in1=wv1_t[:].bitcast(fp32).rearrange("p (h d) -> p h d", d=HD),
                )

            # kt-groups per (j, qb): sizes 3,3,3,3,3,1 (st slot = 3 banks)
            GROUPS = [(0, 3), (3, 3), (6, 3), (9, 3), (12, 3), (15, 1)]

            def emit_scores(j, qb, k0, gn):
                """S^T -> exp for kt in [k0, k0+gn); returns the PV work item."""
                sts, pts = [], []
                for hh in range(2):
                    sts.append(psum.tile([128, 3, 512], fp32, tag="st", name="st"))
                # hh-interleaved emission: consecutive matmuls target disjoint
                # PE row groups (rows 0-63 vs 64-127), so on HW each can start
                # ~4ns after the previous one -> the head pair runs concurrent.
                for u in range(gn):
                    kt = k0 + u
                    for hh in range(2):
                        r0, r1 = hh * HD, (hh + 1) * HD
                        nc.tensor.matmul(
                            sts[hh][:, u, :],
                            (kt_t[(j, kt // 4)][r0:r1, (kt % 4) * 128:(kt % 4 + 1) * 128]),
                            (qt_t[(j, qb)][r0:r1, :]),
                            start=True,
                            stop=True,
                        )
                for hh in range(2):
                    # bufs=3: G5 is scored first and its pts stay live across
                    # the whole qb while the main pipeline double-buffers
                    pt = ptpool.tile([128, 3, 512], bf16, tag="pt", name="pt", bufs=6)
                    pts.append(pt)
                    if os.environ.get("KABL_NOEXP"):
                        nc.vector.tensor_copy(out=pt[:, 0:gn, :], in_=sts[hh][:, 0:gn, :])
                    else:
                        nc.scalar.activation(
                            out=pt[:, 0:gn, :], in_=sts[hh][:, 0:gn, :],
                            func=Exp, scale=0.125,
                        )
                return (pts, k0, gn)

            def emit_pv(j, ctx_ab, work):
                pts, k0, gn = work
                for u in range(gn):
                    kt = k0 + u
                    for hh in range(2):
                        nc.tensor.matmul(
                            ctx_ab[hh][:],
                            (v_t[kt][:, 2 * j + hh, :]),
                            (pts[hh][:, u, :]),
                            start=(kt == 0),
                            stop=(kt == KT_TILES - 1),
                        )

            def emit_attn_epilogue(j, qb, ctx_ab):
                for hh in range(2):
                    rec = epool.tile([1, 512], f32r, tag="rec", name="rec")
                    nc.vector.reciprocal(rec[:], ctx_ab[hh][HD:HD + 1, :])
                    rp = psum.tile([HD, 512], fp32, tag="st", name="st")
                    nc.tensor.matmul(
                        rp[:], (ones[0:1, 0:HD]), (rec[:]), start=True, stop=True
                    )
                    # NB: one DVE op must not read two PSUM operands (single
                    # PSUM port) — stage rp through SBUF.
                    rsb = epool.tile([HD, 512], fp32, tag="rsb", name="rsb")
                    nc.vector.tensor_copy(out=rsb[:], in_=rp[:])
                    nc.vector.tensor_mul(
                        out=ctxn[(j, qb)][hh * HD:(hh + 1) * HD, :],
                        in0=ctx_ab[hh][0:HD, :],
                        in1=rsb[:],
                    )

            def emit_oproj_chunk(qb, qi):
                qt = qb * 4 + qi
                ot = epool.tile([128, 2, 512], fp32, tag="ot", name="ot")
                for nb in range(2):
                    ps = psum.tile([128, 512], fp32, tag="st", name="st")
                    for j in range(2):
                        nc.tensor.matmul(
                            ps[:],
                            (ctxn[(j, qb)][:, qi * 128:(qi + 1) * 128]),
                            (wo_t[j][:, nb * 512:(nb + 1) * 512]),
                            start=(j == 0),
                            stop=(j == 1),
                        )
                    # bias (0.25*bo, rows identical) rides the DVE evacuation
                    nc.vector.tensor_add(
                        out=ot[:, nb, :], in0=ps[:],
                        in1=wo1_t[:, nb * 512:(nb + 1) * 512].bitcast(fp32),
                    )
                # one [128, 1024] store per chunk instead of two halves
                if not os.environ.get("KABL_NOOUT"):
                    nc.sync.dma_start(
                        outp[qt * 128:(qt + 1) * 128, :],
                        ot.rearrange("p a b -> p (a b)"),
                    )

            # ---- emission schedule ----
            # Software-pipelined: PV lags scores+exp by one kt-group so the PE
            # always has independent work (next scores, projections, oproj
            # chunks) in its stream while the scalar engine runs exp.
            def alloc_ctx():
                # the first PV matmul (kt==0) opens the accumulation group
                # with start=True; surplus waits are NOP-split by
                # _split_matmul_waits.
                return [psum.tile([HD + 1, 512], fp32, tag="ctx", name="ctx")
                        for _ in range(2)]

            def lb_parts(j, lb, with_v):
                """Projection work for one L-chunk as filler thunks."""
                xt_blk = alloc_xt_block(lb)
                parts = [
                    lambda: emit_qk_group(kt_t[(j, lb)], wk_t, wk1, j, xt_blk),
                    lambda: emit_qk_group(qt_t[(j, lb)], wq_t, wq1, j, xt_blk),
                ]
                if with_v:
                    for lt in range(lb * 4, lb * 4 + 4):
                        parts.append(lambda lt=lt: emit_v_group(lt, xt_blk))
                return parts

            def attn_qb(j, qb, fillers=()):
                """Pipelined attention for one (j, qb). The tiny tail group
                (G5, one kt) is scored FIRST so its exp is long done when its
                PV runs last (it carries the accumulation stop)."""
                ctx_ab = alloc_ctx()
                fill = list(fillers)
                w_last = emit_scores(j, qb, *GROUPS[5])
                prev = None
                for gi, (k0, gn) in enumerate(GROUPS[:5]):
                    w = emit_scores(j, qb, k0, gn)
                    if prev is not None:
                        emit_pv(j, ctx_ab, prev)
                    if fill:
                        fill.pop(0)()
                    prev = w
                emit_pv(j, ctx_ab, prev)
                for f in fill:
                    f()
                emit_pv(j, ctx_ab, w_last)
                emit_attn_epilogue(j, qb, ctx_ab)

            tail_oproj = []  # last q-bank's oproj, deferred into the next rep
            for _rep in range(reps):
                xt_blocks.clear()
                # (j0, qb0) startup: emit each projection as soon as its own
                # DMAs are in the cumulative count; scores(G_i) depend on K/Q
                # of the lb covering its kt range.
                ctx00 = alloc_ctx()
                ensure_w("k")
                xt0 = alloc_xt_block(0)
                emit_qk_group(kt_t[(0, 0)], wk_t, wk1, 0, xt0)
                # previous rep's tail oproj runs here, under this rep's
                # DMA-bound startup window (steady-state reps only)
                for f in tail_oproj:
                    f()
                tail_oproj = []
                ensure_w("q")
                emit_qk_group(qt_t[(0, 0)], wq_t, wq1, 0, xt0)
                w0 = emit_scores(0, 0, *GROUPS[0])
                ensure_w("v")
                for lt in range(4):
                    emit_v_group(lt, xt0)
                for p in lb_parts(0, 1, True):
                    p()
                w1 = emit_scores(0, 0, *GROUPS[1])
                emit_pv(0, ctx00, w0)
                for p in lb_parts(0, 2, True):
                    p()
                w2 = emit_scores(0, 0, *GROUPS[2])
                emit_pv(0, ctx00, w1)
                w3 = emit_scores(0, 0, *GROUPS[3])
                emit_pv(0, ctx00, w2)
                for p in lb_parts(0, 3, True):
                    p()
                w4 = emit_scores(0, 0, *GROUPS[4])
                emit_pv(0, ctx00, w3)
                w5 = emit_scores(0, 0, *GROUPS[5])
                emit_pv(0, ctx00, w4)
                emit_pv(0, ctx00, w5)
                emit_attn_epilogue(0, 0, ctx00)

                # j=1 projections ride as fillers under the attention stream.
                # K(1,3) must land before attn_qb(1,0) (its G5 scores read it);
                # Q(1,3) is only read by attn_qb(1,3) and fills (1,0).
                k13, q13 = lb_parts(1, 3, False)
                attn_qb(0, 1, fillers=lb_parts(1, 0, False) + [lambda: ensure_w("o")])
                attn_qb(0, 2, fillers=lb_parts(1, 1, False))
                attn_qb(0, 3, fillers=lb_parts(1, 2, False) + [k13])
                attn_qb(1, 0, fillers=[q13])
                for qb in range(1, QB):
                    attn_qb(1, qb,
                            fillers=[lambda qi=qi: emit_oproj_chunk(qb - 1, qi)
                                     for qi in range(4)])
                tail_oproj = [lambda qi=qi: emit_oproj_chunk(QB - 1, qi)
                              for qi in range(4)]
            for f in tail_oproj:
                f()

    if split:
        _split_matmul_waits(nc)
    return nc


def _split_matmul_waits(nc):
    """Walrus allows at most 2 sync commands (waits+updates) per PE matmul.

    Move surplus waits onto same-engine NOPs inserted immediately before
    the instruction (engine streams are in-order, so semantics hold).
    """
    import concourse.mybir as mybir

    SPLIT_KINDS = {
        "InstMatmult", "InstDMACopy", "InstActivation", "InstTensorCopy",
        "InstTensorTensor", "InstMemset", "InstReciprocal", "InstTensorReduce",
        "InstTensorScalar", "InstTensorScalarPtr", "InstCopy", "InstDrain",
    }
    nop_id = 0
    for fn in nc.m.functions:
        for bb in fn.blocks:
            insts = bb.instructions
            out = []
            changed = False
            for inst in insts:
                si = getattr(inst, "sync_info", None)
                kind = type(inst).__name__
                budget_total = 1 if kind in ("InstDrain", "InstNoOp") else 2
                if (
                    kind in SPLIT_KINDS
                    and si is not None
                    and si.on_wait
                    and len(si.on_wait) + len(si.on_update or []) > budget_total
                ):
                    budget = budget_total - len(si.on_update or [])
                    keep = si.on_wait[-budget:] if budget > 0 else []
                    surplus = si.on_wait[: len(si.on_wait) - len(keep)]
                    for w in surplus:
                        nop = mybir.InstNoOp(
                            name=f"I-waitnop{nop_id}",
                            engine=inst.engine,
                            ins=[],
                            outs=[],
                            sync_info=mybir.SyncInfo(on_wait=[w], on_update=[]),
                        )
                        nop_id += 1
                        out.append(nop)
                    inst.sync_info = mybir.SyncInfo(
                        on_wait=keep, on_update=si.on_update
                    )
                    changed = True
                out.append(inst)
            if changed:
                bb.instructions = out
    return nc


def _get_program(split=True):
    global _PROG, _PROG_UNSPLIT
    if split:
        if _PROG is None:
            _PROG = _build_program(split=True)
        return _PROG
    if _PROG_UNSPLIT is None:
        _PROG_UNSPLIT = _build_program(split=False)
    return _PROG_UNSPLIT


def _make_in_maps(x, Wq, bq, Wk, bk, Wv, bv, Wo, bo):
    import ml_dtypes

    f = np.float32
    bf = ml_dtypes.bfloat16
    a = lambda v: np.ascontiguousarray(v, dtype=f)
    ab = lambda v: np.ascontiguousarray(np.asarray(v, dtype=f), dtype=bf)
    in_maps = []
    for c in range(N_CORES):
        b, g = c // 4, c % 4
        s = slice(g * M_LOC, (g + 1) * M_LOC)
        in_maps.append({
            "xta": ab(x[b].T),
            "wqa": ab(Wq[s, :].T), "wqb": a(bq[s].reshape(2, 128).T),
            "wka": ab(Wk[s, :].T), "wkb": a(bk[s].reshape(2, 128).T),
            "wva": ab(Wv[s, :].T), "wvb": a(np.broadcast_to(bv[s][None, :], (128, M_LOC))),
            "woa": ab(Wo[:, s].T), "wob": a(np.broadcast_to(0.25 * bo[None, :], (128, D))),
        })
    return in_maps


def benchmark(reps=15, calls_a=2, calls_b=12, trials=16):
    """Estimate device time per kernel execution.

    One program variant with the per-core computation repeated `reps`
    times inside the NEFF. Timing slope is taken across PIPELINED call
    counts (async dispatch, single block at the end), which cancels the
    multi-ms and highly variable axon per-call overhead that broke the
    two-variant slope. Returns ns per kernel execution.
    """
    import time

    import jax
    from jax.sharding import Mesh, NamedSharding, PartitionSpec
    from jax.experimental.shard_map import shard_map

    import concourse.mybir as mybir
    from concourse import bass2jax

    bass2jax.install_neuronx_cc_hook()

    rng = np.random.default_rng(0)
    fake = dict(
        x=rng.standard_normal((2, L, D)).astype(np.float32),
        Wq=(rng.standard_normal((D, D)) * 0.03).astype(np.float32),
        bq=(rng.standard_normal(D) * 0.01).astype(np.float32),
        Wk=(rng.standard_normal((D, D)) * 0.03).astype(np.float32),
        bk=(rng.standard_normal(D) * 0.01).astype(np.float32),
        Wv=(rng.standard_normal((D, D)) * 0.03).astype(np.float32),
        bv=(rng.standard_normal(D) * 0.01).astype(np.float32),
        Wo=(rng.standard_normal((D, D)) * 0.03).astype(np.float32),
        bo=(rng.standard_normal(D) * 0.01).astype(np.float32),
    )
    in_maps = _make_in_maps(**fake)

    def run_variant(reps):
        nc = _build_program(split=True, reps=reps)

        in_names, out_info = [], []
        pn = nc.partition_id_tensor.name if nc.partition_id_tensor else None
        for alloc in nc.m.functions[0].allocations:
            if not isinstance(alloc, mybir.MemoryLocationSet):
                continue
            name = alloc.memorylocations[0].name
            if alloc.kind == "ExternalInput" and name != pn:
                in_names.append(name)
            elif alloc.kind == "ExternalOutput":
                out_info.append(
                    (name, tuple(alloc.tensor_shape), mybir.dt.np(alloc.dtype))
                )
        out_names = [n for n, _, _ in out_info]
        out_avals = [jax.core.ShapedArray(s, d) for _, s, d in out_info]
        all_in = in_names + out_names + ([pn] if pn else [])

        def _body(*args):
            operands = list(args)
            if pn is not None:
                operands.append(bass2jax.partition_id_tensor())
            return tuple(bass2jax._bass_exec_p.bind(
                *operands,
                out_avals=tuple(out_avals),
                in_names=tuple(all_in),
                out_names=tuple(out_names),
                lowering_input_output_aliases=(),
                sim_require_finite=True,
                sim_require_nnan=True,
                nc=nc,
            ))

        devices = jax.devices()[:N_CORES]
        mesh = Mesh(np.asarray(devices), ("core",))
        nio = len(in_names) + len(out_names)
        f = jax.jit(shard_map(
            _body, mesh=mesh,
            in_specs=(PartitionSpec("core"),) * nio,
            out_specs=(PartitionSpec("core"),) * len(out_names),
            check_rep=False,
        ), keep_unused=True)
        sh = NamedSharding(mesh, PartitionSpec("core"))
        args = [
            jax.device_put(
                np.concatenate(
                    [np.asarray(in_maps[c][n]) for c in range(N_CORES)], axis=0
                ),
                sh,
            )
            for n in in_names
        ] + [
            jax.device_put(np.zeros((N_CORES * s[0], *s[1:]), d), sh)
            for _, s, d in out_info
        ]
        jax.block_until_ready(f(*args))

        def timed(ncalls):
            r = None
            t0 = time.perf_counter()
            for _ in range(ncalls):
                r = f(*args)
            jax.block_until_ready(r)
            return time.perf_counter() - t0

        timed(2)  # warm the dispatch path
        slopes = []
        for _ in range(trials):
            ta = timed(calls_a)
            tb = timed(calls_b)
            slopes.append((tb - ta) / (calls_b - calls_a))
        if os.environ.get("BENCH_VERBOSE"):
            print("bench slopes/exec us:",
                  [f"{s / reps * 1e6:.1f}" for s in slopes])
        # axon timing noise can produce nonphysical slopes (even negative);
        # keep only trials within a sane band of the median before min()
        med = sorted(slopes)[len(slopes) // 2]
        valid = [s for s in slopes if s > 0.4 * med]
        return (min(valid) if valid else med) / reps * 1e9

    return run_variant(reps)


def kernel(x, Wq, bq, Wk, bk, Wv, bv, Wo, bo):
    global LAST_RESULTS
    x = np.asarray(x, dtype=np.float32)
    nc = _get_program()
    in_maps = _make_in_maps(
        x, np.asarray(Wq), np.asarray(bq), np.asarray(Wk), np.asarray(bk),
        np.asarray(Wv), np.asarray(bv), np.asarray(Wo), np.asarray(bo),
    )

    if os.environ.get("BASS_KERNEL_SIM"):
        from concourse.bass_interp import CoreSim

        nc = _get_program(split=False)
        results = []
        for c in range(int(os.environ.get("BASS_KERNEL_SIM_CORES", N_CORES))):
            sim = CoreSim(nc)
            for name, val in in_maps[c].items():
                sim.tensor(name)[:] = val
            sim.simulate()
            results.append({"outp": np.array(sim.tensor("outp"))})
    else:
        from concourse import bass2jax

        results = bass2jax.run_bass_via_pjrt(nc, in_maps, n_cores=N_CORES)

    B = x.shape[0]
    out = np.stack([
        np.sum([results[4 * b + g]["outp"] for g in range(4)], axis=0)
        for b in range(B)
    ]).astype(np.float32)
    return out

